# revision 64
# baseline (speedup 1.0000x reference)
"""Trainium2 Bass kernel: multi-head attention (1x1-conv K/V projections,
per-head GhostBatchNorm eval-mode affine, key+query masking, softmax).

Host: batch data-parallel over 8 cores (2 batches/core); mask compaction
with ASYMMETRIC slot pairing: batches sorted by valid count and paired
largest-with-smallest, so slot 0 pads to 544 (5 key chunks, max valid 543)
while slot 1 pads to 512 (4 chunks, no 32-wide query tails, both heads'
scores in ONE 2-bank PSUM tile with a single merged exp per step).  GBN
scale folded into q; V bias folded into host epilogue (out = num/den +
v_b); softmax division on host.

Device (per core):
  - K proj fp32r (feeds exp-amplified scores), V proj bf16 (bias removed:
    it cancels into the host epilogue via the denominator).  Projections
    feed into the attention steps (full groups early, half-groups late) to
    keep the PE dense for the HAM clock governor.
  - Per-key exp bias from the mask (-45 valid / -150 padding) makes es of
    padding keys exactly 0, so the vpv mask multiply is unnecessary and the
    PV denominator row is plain ones.
  - Attention per head-pair: per-head [sl,544] score PSUM tiles (2-bank
    slots, double-buffered), scores one i-step ahead of exp/PV; PV
    accumulates both heads in one [65,1088] PSUM tile (3 banks; pieces
    512/32/480/64 with has_written bank-clear pattern).  Row 64 of PV is
    the softmax denominator.
  - Evacuate [65,544] per head (h0 DVE, h1 ACT) and DMA out; host divides.
"""

import numpy as np

BS, DA, SL, H = 16, 512, 1024, 8
N_CORES = 8
B = BS // N_CORES
P = 128
NT = DA // P       # channel tiles (4)
DH = DA // H       # head dim (64)

SPAD = 544         # DRAM-padded sequence length (slot-0 batches)
MPAD = 640         # mask-bias padding (5 x 128 for the [P,5] layout)
NSP = 5            # max s-chunks
S_OFF = [0, 128, 256, 384, 512]
QPAD = SPAD
# Per-slot asymmetry: the host pairs each core's largest-valid batch (slot 0,
# pad 544, 5 key chunks) with a small one (slot 1, pad 512, 4 chunks) so
# slot 1 needs no 5th chunk, no 32-wide query tails, and 8 fewer exps.
SPADS = (544, 512)
NSPS = (5, 4)
S_LENS = ([128, 128, 128, 128, 32], [128, 128, 128, 128])
QPS = (((0, 512), (512, 32)), ((0, 512),))
_CACHE: dict = {}


def build_nc(n_batches=B, n_pairs=H // 2):
    from contextlib import ExitStack

    import concourse.bass as bass
    import concourse.tile as tile
    from concourse import bacc, mybir

    dt = mybir.dt.float32
    dtr = mybir.dt.float32r
    bf16 = mybir.dt.bfloat16
    Act = mybir.ActivationFunctionType

    nc = bacc.Bacc("TRN2", target_bir_lowering=False, debug=False)

    q_d = nc.dram_tensor("q", [n_batches, DA, SPAD], dtr, kind="ExternalInput")
    kin_d = nc.dram_tensor("k_in", [n_batches, DA, SPAD], dtr, kind="ExternalInput")
    vin_d = nc.dram_tensor("v_in", [n_batches, DA, SPAD], bf16, kind="ExternalInput")
    kwT_d = nc.dram_tensor("k_wT", [DA, DA], dtr, kind="ExternalInput")
    vwT_d = nc.dram_tensor("v_wT", [DA, DA], bf16, kind="ExternalInput")
    # k_b and the mask bias ship pre-transposed so each partition's slice is
    # contiguous (otherwise the DMA degenerates to 4-byte descriptors and
    # costs ~2us of serial descriptor generation on the Sync engine)
    kb_d = nc.dram_tensor("k_bT", [P, NT], dt, kind="ExternalInput")
    mf_d = nc.dram_tensor("maskbT", [n_batches, P, NSP], dt, kind="ExternalInput")
    out_d = nc.dram_tensor(
        "out", [n_batches, H, DH + 1, QPAD], bf16, kind="ExternalOutput"
    )

    with tile.TileContext(nc) as tc:
        with ExitStack() as ctx:
            consts = ctx.enter_context(tc.tile_pool(name="consts", bufs=1))
            qpool = ctx.enter_context(tc.tile_pool(name="qpool", bufs=2))
            kinp = ctx.enter_context(tc.tile_pool(name="kinp", bufs=2))
            vinp = ctx.enter_context(tc.tile_pool(name="vinp", bufs=2))
            mpool = ctx.enter_context(tc.tile_pool(name="mpool", bufs=2))
            kspool = ctx.enter_context(tc.tile_pool(name="kspool", bufs=2))
            vpvpool = ctx.enter_context(tc.tile_pool(name="vpvpool", bufs=2))
            epool = ctx.enter_context(tc.tile_pool(name="epool", bufs=3))
            orpool = ctx.enter_context(tc.tile_pool(name="orpool", bufs=6))
            pscore = ctx.enter_context(tc.tile_pool(name="pscore", bufs=2, space="PSUM"))
            ppv = ctx.enter_context(tc.tile_pool(name="ppv", bufs=1, space="PSUM"))
            pproj = ctx.enter_context(tc.tile_pool(name="pproj", bufs=1, space="PSUM"))

            # ---- tile allocations (DMAs are issued separately, in
            # dependency order: the input DMA ring drains serially at HBM
            # rate, so issue order IS the load schedule) ----
            kwT_sb = consts.tile([P, NT, DA], dtr)  # [p, ci, o]; c = ci*128+p
            vwT_sb = consts.tile([P, NT, DA], bf16)
            kb_col = consts.tile([P, NT], dt)  # k_b[o]; o = t*128+p
            warm_sb = consts.tile([P, 640], bf16)
            nc.vector.memset(warm_sb[:], 0.5)

            tiles = {}

            def alloc_batch(b):
                kin_sb = kinp.tile([P, NT, SPAD], dtr, name=f"kin{b}", tag="kin")
                vin_sb = vinp.tile([P, NT, SPAD], bf16, name=f"vin{b}", tag="vin")
                q_sb = qpool.tile([P, NT, SPAD], dtr, name=f"q{b}", tag="q")
                maskb = mpool.tile([P, NSP], dt, name=f"mb{b}", tag="mb")
                k_sb = kspool.tile([P, NT, SPAD], dtr, name=f"ks{b}", tag="ks")
                v_pv = vpvpool.tile(
                    [P, NSP, H, DH + 1], bf16, name=f"vpv{b}", tag="vpv"
                )
                # the 65th lhsT column: plain ones (padding keys are killed
                # by the exp bias, not here)
                nc.vector.memset(v_pv[:, :, :, DH], 1.0)
                tiles[b] = dict(
                    kin=kin_sb, vin=vin_sb, q=q_sb, mb=maskb, ks=k_sb, vpv=v_pv
                )

            alloc_batch(0)
            alloc_batch(1)

            def dma_kwT(t_lo, t_hi):
                nc.sync.dma_start(
                    out=kwT_sb[:, :, t_lo * P : t_hi * P],
                    in_=kwT_d.ap()[:, t_lo * P : t_hi * P].rearrange(
                        "(ci p) o -> p ci o", p=P
                    ),
                )

            def dma_q(b, t_lo, t_hi):
                nc.sync.dma_start(
                    out=tiles[b]["q"][:, t_lo:t_hi, :],
                    in_=q_d.ap()[b][t_lo * P : t_hi * P].rearrange(
                        "(t p) s -> p t s", p=P
                    ),
                )

            # batch-0 critical path: K t0 -> V i0 -> scores(pr0).  Tiny
            # transfers (kb, mask bias) go first; the ring drains serially
            # at HBM rate so this order IS the arrival schedule.
            dma_kwT(0, 1)
            nc.sync.dma_start(out=kb_col[:], in_=kb_d.ap())
            nc.sync.dma_start(out=tiles[0]["mb"][:], in_=mf_d.ap()[0])
            nc.sync.dma_start(
                out=tiles[0]["kin"][:],
                in_=kin_d.ap()[0].rearrange("(t p) s -> p t s", p=P),
            )
            nc.sync.dma_start(
                out=vwT_sb[:], in_=vwT_d.ap().rearrange("(ci p) o -> p ci o", p=P)
            )
            nc.sync.dma_start(
                out=tiles[0]["vin"][:],
                in_=vin_d.ap()[0].rearrange("(t p) s -> p t s", p=P),
            )
            dma_q(0, 0, 1)
            dma_kwT(1, 2)
            dma_q(0, 1, 4)
            dma_kwT(2, 4)
            nc.sync.dma_start(
                out=tiles[1]["kin"][:],
                in_=kin_d.ap()[1].rearrange("(t p) s -> p t s", p=P),
            )
            nc.sync.dma_start(
                out=tiles[1]["vin"][:],
                in_=vin_d.ap()[1].rearrange("(t p) s -> p t s", p=P),
            )
            dma_q(1, 0, 4)
            nc.sync.dma_start(out=tiles[1]["mb"][:], in_=mf_d.ap()[1])

            # preload the exp table on ACT during the DMA drain so the first
            # real exp doesn't pay the ~1.4us ACT_TABLE_LOAD
            dume = consts.tile([P, 2], dt)
            nc.vector.memset(dume[:], 0.0)
            nc.scalar.activation(
                dume[:, 1:2], dume[:, 0:1], Act.Exp, bias=dume[:, 0:1]
            )

            proj_state: dict = {}

            def emit_proj_half(b, g, half, slot=None):
                """g 0..7: K proj (t=g//2, piece=g%2); g 8..12: V proj
                (i=g-8).  half 0 = contract tiles 0-1, half 1 = 2-3 + evac."""
                t_b = tiles[b]
                key = (b, g)
                if half == 0:
                    if slot is None:
                        slot = pproj.tile([P, 512], dt, tag="proj", name=f"p{b}_{g}")
                    proj_state[key] = slot
                else:
                    slot = proj_state.pop(key)
                cis = (0, 1) if half == 0 else (2, 3)
                if g < 8:
                    t, piece = g // 2, g % 2
                    qo, nq = QPS[b][piece]
                    kp = slot[:, 0:nq]
                    for ci in cis:
                        nc.tensor.matmul(
                            kp,
                            kwT_sb[:, ci, t * P : (t + 1) * P],
                            t_b["kin"][:, ci, qo : qo + nq],
                            start=(ci == 0),
                            stop=(ci == NT - 1),
                        )
                    if half == 1:
                        nc.vector.tensor_scalar_add(
                            t_b["ks"][:, t, qo : qo + nq], kp, kb_col[:, t : t + 1]
                        )
                else:
                    i = g - 8
                    so, sl = S_OFF[i], S_LENS[b][i]
                    vp = slot[0:sl, 0:DA]
                    for ci in cis:
                        nc.tensor.matmul(
                            vp,
                            t_b["vin"][:, ci, so : so + sl],
                            vwT_sb[:, ci, :],
                            start=(ci == 0),
                            stop=(ci == NT - 1),
                        )
                    if half == 1:
                        nc.vector.tensor_copy(
                            t_b["vpv"][0:sl, i, :, 0:DH],
                            vp.rearrange("p (h d) -> p h d", h=H),
                        )

            def emit_proj_group(b, g, slot):
                emit_proj_half(b, g, 0, slot)
                emit_proj_half(b, g, 1)

            # PV pieces per slot: (col, n, es_off, may_start).  Slot 0's
            # merged [65,1088] pair tile spans banks A=0-511, B=512-1023,
            # C=1024-1087; h0's tail (512,32) clears bank B at i=0 and h1's
            # 480-piece rides it with start=False.  Slot 1 (512-pad) is two
            # clean bank-sized pieces.
            PV_PIECES = (
                {
                    0: ((0, 512, 0, True), (512, 32, 512, True)),
                    1: ((544, 480, 0, False), (1024, 64, 480, True)),
                },
                {
                    0: ((0, 512, 0, True),),
                    1: ((512, 512, 0, True),),
                },
            )

            def emit_scores(b, pr, i):
                t_b = tiles[b]
                so, sl = S_OFF[i], S_LENS[b][i]
                if b == 1:
                    # slot 1 is 512/head: both heads fit one 2-bank tile
                    # (same slot size as slot 0's per-head tiles), so one
                    # slot serves the whole step -> one merged exp and a
                    # real two-step score pipeline
                    sc_m = pscore.tile([P, 1024], dt, tag="sch", name="scm")
                    scs = [sc_m[:, 0:512], sc_m[:, 512:1024]]
                else:
                    sc_m = None
                    scs = [
                        pscore.tile([P, QPAD], dt, tag="sch", name=f"sc{hh}")
                        for hh in range(2)
                    ]
                # piece-major emission keeps the two heads' matmuls adjacent
                # in the PE queue so row-group tiling runs them concurrently
                for (qo, nq) in QPS[b]:
                    for hh in range(2):
                        nc.tensor.matmul(
                            scs[hh][0:sl, qo : qo + nq],
                            t_b["ks"][hh * 64 : (hh + 1) * 64, pr, so : so + sl],
                            t_b["q"][hh * 64 : (hh + 1) * 64, pr, qo : qo + nq],
                            start=True,
                            stop=True,
                        )
                return scs, sc_m

            pv_state = {"pv": None}

            def emit_pv(b, pr, i, ess):
                t_b = tiles[b]
                sl = S_LENS[b][i]
                if i == 0:
                    pv_state["pv"] = ppv.tile([65, 1088], dt, name="pv", tag="pv")
                pv = pv_state["pv"]
                for hh in range(2):
                    lhsT = t_b["vpv"][0:sl, i, 2 * pr + hh, :]
                    for (co, nq, eo, may_start) in PV_PIECES[b][hh]:
                        nc.tensor.matmul(
                            pv[0:65, co : co + nq],
                            lhsT,
                            ess[hh][0:sl, eo : eo + nq],
                            start=(i == 0 and may_start),
                            stop=(i == NSPS[b] - 1),
                        )

            def emit_out(b, pr, last=False):
                # evacuate (bf16 cast) + ship; host divides.  Both copies on
                # DVE: ACT is the per-step pacer now, and an ACT-side copy
                # queues behind the next step's exp, delaying the pv-slot
                # WAR release by ~1.9us at every pair boundary
                pv = pv_state["pv"]
                qp = SPADS[b]
                for hh in range(2):
                    h = 2 * pr + hh
                    o_raw = orpool.tile(
                        [65, QPAD], bf16, name=f"oraw{b}_{h}", tag="oraw"
                    )
                    if last and hh == 1:
                        # final pair: no later exp exists, so ACT is idle
                        # and the two copies run in parallel, shortening
                        # the drain critical path
                        nc.scalar.activation(
                            o_raw[:, 0:qp], pv[0:65, qp : 2 * qp], Act.Copy
                        )
                    else:
                        nc.vector.tensor_copy(
                            o_raw[:, 0:qp], pv[0:65, hh * qp : (hh + 1) * qp]
                        )
                    nc.sync.dma_start(
                        out=out_d.ap()[b, h][:, 0:qp], in_=o_raw[:, 0:qp]
                    )

            def attention_stream(steps, step_feed):
                """One flat software-pipelined stream over (b, pr, i) steps.
                Scores run one step ahead of exp; PV runs one step BEHIND
                exp (its es is always ready), so the PE queue never stalls
                on the ACT engine.  step_feed maps step idx -> list of
                (b, g, half) projection emissions."""
                scs, scm = emit_scores(*steps[0])
                for idx, (b, pr, i) in enumerate(steps):
                    t_b = tiles[b]
                    sl = S_LENS[b][i]
                    qp = SPADS[b]
                    if scm is not None:
                        es_m = epool.tile([P, 1024], bf16, name="esm", tag="e0")
                        nc.scalar.activation(
                            es_m[0:sl, :],
                            scm[0:sl, :],
                            Act.Exp,
                            bias=t_b["mb"][0:sl, i : i + 1],
                        )
                        ess = [es_m[:, 0:512], es_m[:, 512:1024]]
                    else:
                        ess = []
                        for hh in range(2):
                            es = epool.tile(
                                [P, QPAD], bf16, name=f"es{hh}", tag=f"e{hh}"
                            )
                            nc.scalar.activation(
                                es[0:sl, 0:qp],
                                scs[hh][0:sl, 0:qp],
                                Act.Exp,
                                bias=t_b["mb"][0:sl, i : i + 1],
                            )
                            ess.append(es)
                    if idx + 1 < len(steps):
                        scs, scm = emit_scores(*steps[idx + 1])
                    emit_pv(b, pr, i, ess)
                    if i == NSPS[b] - 1:
                        emit_out(b, pr, last=(idx == len(steps) - 1))
                    for fb, fg, fh in step_feed.get(idx, ()):
                        emit_proj_half(fb, fg, fh)

            # ================= emission =================
            # batch-0 preamble (overlaps the startup DMA drain): warmup
            # bursts keep the PE busy between DMA arrivals and ramp the HAM
            # clock gate; K t0 / V i0 / K t1 rotate over the proj bank and
            # the two score slots so evacuations pipeline without PSUM
            # write-after-read stalls
            scp0 = pscore.tile([P, QPAD], dt, tag="sch", name="scp0")
            scp1 = pscore.tile([P, QPAD], dt, tag="sch", name="scp1")
            pvp = ppv.tile([P, 1024], dt, tag="pv", name="pvp")

            def warm(n, region):
                for _ in range(n):
                    nc.tensor.matmul(
                        region,
                        warm_sb[:, 0:P],
                        warm_sb[:, P : P + 512],
                        start=True,
                        stop=True,
                    )

            warm_ps = pproj.tile([P, 512], dt, tag="proj", name="warmup")
            for j in range(10):
                warm(1, pvp[:, 0:512] if j % 2 else pvp[:, 512:1024])
            emit_proj_group(0, 0, warm_ps[:, 0:512])  # K t0 512-piece
            # ks-pinned bridge fillers: they read ks (K t0's evacuation), so
            # the scheduler cannot hoist them before it; they fill the PE
            # idle window while vin0/q0 are still in the DMA ring, keeping
            # the HAM activity monitor from demoting the clock mid-startup
            for _ in range(4):
                nc.tensor.matmul(
                    pvp[0:P, 0:512],
                    tiles[0]["ks"][0:64, 0, 0:P],
                    tiles[0]["kin"][0:64, 0, 0:512],
                    start=True,
                    stop=True,
                )
            emit_proj_group(0, 8, scp0[:, 0:512])     # V i0

            # deadline-aware half-group feed: V i_k of a batch is needed at
            # that batch's step k; K t_k by step 5k-1 (scores emit one step
            # ahead).  Batch 0's V i1..i4 must front-load; everything else
            # spreads one half-group per step to keep PE dense but never
            # ahead of ACT.
            step_feed = {
                0: [(0, 9, 0), (0, 9, 1), (0, 2, 0), (0, 2, 1)],
                1: [(0, 10, 0), (0, 10, 1), (0, 1, 0), (0, 1, 1)],
                2: [(0, 11, 0), (0, 11, 1), (0, 3, 0), (0, 3, 1)],
                3: [(0, 12, 0), (0, 12, 1)],
            }
            feed_full = [
                (0, 4), (0, 5), (0, 6), (0, 7), (1, 0), (1, 8), (1, 9),
                (1, 2), (1, 10), (1, 11),
            ]
            feed_half = [(1, 4), (1, 6)]
            for s, (b, g) in enumerate(feed_full):
                step_feed[4 + s] = [(b, g, 0), (b, g, 1)]
            base = 4 + len(feed_full)
            for j, (b, g) in enumerate(feed_half):
                step_feed[base + 2 * j] = [(b, g, 0)]
                step_feed[base + 2 * j + 1] = [(b, g, 1)]
            if n_batches == 1:
                step_feed = {
                    k: [e for e in v if e[0] == 0] for k, v in step_feed.items()
                }
            steps = [
                (b, pr, i)
                for b in range(n_batches)
                for pr in range(n_pairs)
                for i in range(NSPS[b])
            ]
            attention_stream(steps, step_feed)

    nc.compile()
    return nc


def _get_nc():
    if "nc" not in _CACHE:
        _CACHE["nc"] = build_nc()
    return _CACHE["nc"]


def _prepare(inputs):
    """Host-side compaction + sharding.  Returns (in_maps, keep_idx list)."""
    q = np.asarray(inputs["q"], dtype=np.float32)
    k_in = np.asarray(inputs["k_in"], dtype=np.float32)
    v_in = np.asarray(inputs["v_in"], dtype=np.float32)
    k_w = np.asarray(inputs["k_w"], dtype=np.float32)
    k_b = np.asarray(inputs["k_b"], dtype=np.float32)
    v_w = np.asarray(inputs["v_w"], dtype=np.float32)
    gamma = np.asarray(inputs["gbn_gamma"], dtype=np.float32)
    gs = np.asarray(inputs["gbn_s"], dtype=np.float32)
    mask = np.asarray(inputs["mask"]).reshape(BS, SL)

    a = (gamma / gs).astype(np.float32)
    q_scaled = (
        (q.reshape(BS, H, DH, SL) * a[None, :, None, None]).reshape(BS, DA, SL)
    ).astype(np.float32)

    keeps = [np.flatnonzero(mask[b] == 0) for b in range(BS)]
    ns = np.array([len(k) for k in keeps])
    # pair each core's largest batch (slot 0, 544-pad, 5 chunks) with a
    # small one (slot 1, 512-pad, 4 chunks): sort desc, pair ends inward
    order = np.argsort(-ns, kind="stable")
    pairs = [(int(order[c]), int(order[BS - 1 - c])) for c in range(N_CORES)]
    for c, (g0, g1) in enumerate(pairs):
        if ns[g0] > SPADS[0] or ns[g1] > SPADS[1]:
            raise ValueError(f"core {c}: ns {ns[g0]},{ns[g1]} exceed pads {SPADS}")

    qc = np.zeros((BS, DA, SPAD), np.float32)
    kc = np.zeros((BS, DA, SPAD), np.float32)
    vc = np.zeros((BS, DA, SPAD), np.float32)
    mb = np.full((BS, MPAD), -150.0, np.float32)   # exp bias: padding -> es=0
    for b, kidx in enumerate(keeps):
        n = len(kidx)
        qc[b, :, :n] = q_scaled[b][:, kidx]
        kc[b, :, :n] = k_in[b][:, kidx]
        vc[b, :, :n] = v_in[b][:, kidx]
        mb[b, :n] = -45.0

    k_wT = np.ascontiguousarray(k_w.T, dtype=np.float32)
    v_wT = np.ascontiguousarray(v_w.T, dtype=np.float32)
    k_bT = np.ascontiguousarray(k_b.reshape(NT, P).T)        # [p, t]
    mbT = np.ascontiguousarray(
        mb[:, : NSP * P].reshape(BS, NSP, P).transpose(0, 2, 1)
    )                                                        # [b, p, i]

    def b16(x):
        import ml_dtypes

        return np.asarray(x, dtype=ml_dtypes.bfloat16)

    in_maps = []
    for c in range(N_CORES):
        gsel = list(pairs[c])
        in_maps.append(
            {
                "q": np.ascontiguousarray(qc[gsel]),
                "k_in": np.ascontiguousarray(kc[gsel]),
                "v_in": b16(np.ascontiguousarray(vc[gsel])),
                "k_wT": k_wT,
                "v_wT": b16(v_wT),
                "k_bT": k_bT,
                "maskbT": np.ascontiguousarray(mbT[gsel]),
            }
        )
    return in_maps, (keeps, pairs)


def _scatter(results, meta, v_b) -> np.ndarray:
    keeps, pairs = meta
    out = np.zeros((BS, DA, SL), np.float32)
    for c in range(N_CORES):
        oc = np.asarray(results[c]["out"], dtype=np.float32)  # [B,H,DH+1,QPAD]
        for s in range(B):
            b = pairs[c][s]
            kidx = keeps[b]
            n = len(kidx)
            num = oc[s, :, :DH, :n]                   # [H, 64, n]
            den = oc[s, :, DH, :n]                    # [H, n]
            out[b][:, kidx] = (num / den[:, None, :]).reshape(DA, n) + v_b[:, None]
    return out


def kernel(**inputs) -> np.ndarray:
    from concourse.bass_utils import run_bass_kernel_spmd

    in_maps, meta = _prepare(inputs)
    v_b = np.asarray(inputs["v_b"], dtype=np.float32)
    nc = _get_nc()
    res = run_bass_kernel_spmd(nc, in_maps, list(range(N_CORES)))
    return _scatter(res.results, meta, v_b)
